# revision 43
# baseline (speedup 1.0000x reference)
"""Causal attention (QKV proj + softmax(QK^T/sqrt(d))V) on 8 TRN2 NeuronCores.

Sharding: data-parallel over batch (B=8, one batch element per core).
Host-side input prep (part of kernel()'s sharding step): x is cast to
bf16 and transposed to x^T [D,T]; Wq/Wk/Wv are cast to bf16 and laid
out so every weight tile is a single contiguous DMA. All matmuls are
bf16: 1 cyc/col PE streaming + FWL fast weight load (f32r streams at
the same rate but its 4-byte LDWEIGHTS can't use FWL and stays
exposed; fp16 measured slower than bf16 on the MATMUL stream).

Per-core kernel:
  phase 1: per 512-wide t-slice: Q then K projections (weights fully
           resident in SBUF), evicted to resident Q^T / K^T tiles (no
           DRAM roundtrip); then V = x @ Wv into a resident V tile.
  phase 2: per 512-wide query supertile: S^T = K Q^T (probs produced
           directly in the lhsT layout P@V needs), diagonal band
           trimmed to the causal range, exp on ACT with fused
           1/sqrt(D) scale, 128x128 causal mask on the diagonal block
           only, P@V with interleaved ones-matmul row sums on PE,
           reciprocal normalize, store.

DMA queues (learned the hard way): sync(HWDGE) = x^T loads - latency
critical; gpsimd(SWDGE) = weight loads - bulk, latency tolerant;
scalar(HWDGE) = a few early x^T assists + output stores.
"""

import numpy as np

T = 2048
D = 1024
E = 1024
N_CORES = 8
P = 128
TS = 512  # t-slice / supertile width
SCALE = 1.0 / 32.0  # 1/sqrt(D)

DC = D // P  # 8 d-chunks
EC = E // P  # 8 e-chunks
TB = T // P  # 16 t-blocks of 128
NTS = T // TS  # 4 t-slices of 512
JB = TS // P  # 4 q-blocks per supertile
QB = TB // 4  # pT quarter size in k-blocks
EH = E // TS  # 2 e-halves


def _attention_kernel(ctx, tc, out, xt, wqp, wkp, wvp):
    from concourse import mybir
    from concourse.bass import ts

    nc = tc.nc
    f32 = mybir.dt.float32
    bf16 = mybir.dt.bfloat16
    AF = mybir.ActivationFunctionType

    # ---- left-side SBUF pools ----
    const = ctx.enter_context(tc.tile_pool(name="const", bufs=1))
    ones_f32 = const.tile([P, 2], f32)
    nc.vector.memset(ones_f32[:], 1.0)
    ones_col = const.tile([P, 2], bf16)
    nc.vector.tensor_copy(ones_col[:], ones_f32[:])
    # warm the ACT exp table set at program start (off the critical path)
    exp_warm = const.tile([P, 2], f32)
    nc.scalar.activation(exp_warm[:], ones_f32[:], AF.Exp)

    kt_pool = ctx.enter_context(tc.tile_pool(name="ktres", bufs=1))
    KT = kt_pool.tile([P, EC, T], bf16)  # K^T[e, t], e = ec*128 + ep
    qt_pool = ctx.enter_context(tc.tile_pool(name="qtres", bufs=1))
    QT = qt_pool.tile([P, EC, T], bf16)  # Q^T[e, t], resident

    # ---- right-side work pools ----
    tc.swap_default_side()
    xv_pool = ctx.enter_context(tc.tile_pool(name="xv", bufs=1))
    xv = xv_pool.tile([P, DC, T], bf16)  # x^T[d, t]: [dp, dc, t]
    v_pool = ctx.enter_context(tc.tile_pool(name="vres", bufs=1))
    V = v_pool.tile([P, TB, E], bf16)  # V[t, e]: [tl, tb, e]
    wres_pool = tc.alloc_tile_pool(name="wres", bufs=16)
    wvh_pool = tc.alloc_tile_pool(name="wvh", bufs=2)
    tc.swap_default_side()

    # ---- PSUM pool for phase 1 ----
    ps_proj = tc.alloc_tile_pool(name="ps_proj", bufs=8, space="PSUM")

    # ---- x^T streaming loads: per (t-slice, d-chunk), sync queue ----
    # (consumption order; first Q matmul can start after 8 chunks)
    def load_xt_tslice(tsl):
        for dc in range(DC):
            eng = nc.scalar if (tsl == 0 and dc % 2 == 1) else nc.sync
            eng.dma_start(
                xv[:, dc, ts(tsl, TS)], xt[ts(dc, P), ts(tsl, TS)]
            )

    load_xt_tslice(0)

    # ---- resident bf16 weight tiles (single contiguous DMAs) ----
    wr_q, wr_k = [], []
    for w_ap, lst, nm in ((wqp, wr_q, "q"), (wkp, wr_k, "k")):
        for eb in range(EC):
            wr = wres_pool.tile(
                [P, DC, P], bf16, tag="wres", name=f"wr{nm}_{eb}"
            )
            # first Q tiles ride the scalar HWDGE queue: SWDGE's first-op
            # latency would stall the opening projection chains
            weng = nc.scalar if (nm == "q" and eb < 3) else nc.gpsimd
            weng.dma_start(wr[:], w_ap[:, eb])
            lst.append(wr)
    wvhs = []
    for eh in range(EH):
        wvh = wvh_pool.tile([P, DC, TS], bf16, tag="wvh", name=f"wvh_{eh}")
        nc.gpsimd.dma_start(wvh[:], wvp[:, eh])
        wvhs.append(wvh)

    for tsl in range(1, NTS):
        load_xt_tslice(tsl)

    # 128x128 causal mask for the diagonal blocks of S^T: keep f >= p
    # (p = key partition, f = query col within the block).
    mask_pool = ctx.enter_context(tc.tile_pool(name="maskp", bufs=1))
    mask_f32 = mask_pool.tile([P, P], f32)
    nc.vector.memset(mask_f32[:], 1.0)
    nc.gpsimd.affine_select(
        out=mask_f32[:],
        in_=mask_f32[:],
        compare_op=mybir.AluOpType.is_ge,
        fill=0.0,
        base=0,
        pattern=[[1, P]],
        channel_multiplier=-1,
    )
    mask128 = mask_pool.tile([P, P], bf16)
    nc.vector.tensor_copy(mask128[:], mask_f32[:])


    # ===== phase 1a: per t-slice, Q then K projections =====
    for tsl in range(NTS):
        for wr_lst, dst in ((wr_q, QT), (wr_k, KT)):
            for eb in range(EC):
                pp = ps_proj.tile([P, TS], f32)
                for dc in range(DC):
                    nc.tensor.matmul(
                        pp[:],
                        wr_lst[eb][:, dc, :],
                        xv[:, dc, ts(tsl, TS)],
                        start=(dc == 0),
                        stop=(dc == DC - 1),
                    )
                if eb % 2 == 0:
                    nc.vector.tensor_copy(dst[:, eb, ts(tsl, TS)], pp[:])
                else:
                    nc.scalar.copy(dst[:, eb, ts(tsl, TS)], pp[:])

    # ===== phase 1b: V = x @ Wv into the resident V tile =====
    for tb in range(TB):
        for eh in range(EH):
            pp = ps_proj.tile([P, TS], f32)
            for dc in range(DC):
                nc.tensor.matmul(
                    pp[:],
                    xv[:, dc, ts(tb, P)],
                    wvhs[eh][:, dc, :],
                    start=(dc == 0),
                    stop=(dc == DC - 1),
                )
            if eh == 0:
                nc.vector.tensor_copy(V[:, tb, ts(eh, TS)], pp[:])
            else:
                nc.scalar.copy(V[:, tb, ts(eh, TS)], pp[:])

    wvh_pool.release()
    wres_pool.release()
    ps_proj.release()

    # ================= phase 2: attention =================
    ps_s = tc.alloc_tile_pool(name="ps_s", bufs=4, space="PSUM")
    ps_o = tc.alloc_tile_pool(name="ps_o", bufs=2, space="PSUM")
    ps_sum = tc.alloc_tile_pool(name="ps_sum", bufs=2, space="PSUM")

    tc.swap_default_side()
    pt_pool = ctx.enter_context(tc.tile_pool(name="pt", bufs=5))
    rs_pool = ctx.enter_context(tc.tile_pool(name="rs", bufs=8))
    ostg = ctx.enter_context(tc.tile_pool(name="ostg", bufs=3))
    tc.swap_default_side()

    for sup in range(NTS):
        nkb = JB * sup + JB  # key blocks 0..nkb-1
        pt_parts = [
            pt_pool.tile([P, QB, TS], bf16, tag="pt", name=f"ptp_{sup}_0")
        ]

        # --- S^T blocks + exp + causal mask (diagonal band trimmed) ---
        for k in range(nkb):
            j = k - JB * sup  # >= 0 on the diagonal band
            lo = max(0, j) * P  # first causal query col in this supertile
            ssp = ps_s.tile([P, TS], f32)
            for ec in range(EC):
                nc.tensor.matmul(
                    ssp[:, lo:TS],
                    KT[:, ec, ts(k, P)],
                    QT[:, ec, sup * TS + lo : (sup + 1) * TS],
                    start=(ec == 0),
                    stop=(ec == EC - 1),
                )
            if k // QB >= len(pt_parts):
                pt_parts.append(
                    pt_pool.tile(
                        [P, QB, TS], bf16, tag="pt",
                        name=f"ptp_{sup}_{k // QB}",
                    )
                )
            pk = pt_parts[k // QB][:, k % QB, :]
            nc.scalar.activation(
                pk[:, lo:TS], ssp[:, lo:TS], AF.Exp, scale=SCALE
            )
            if j >= 0:
                nc.vector.tensor_mul(
                    pk[:, lo : lo + P], pk[:, lo : lo + P], mask128[:]
                )

        # --- P @ V (+ row sums interleaved in eh=0), normalize, store ---
        rss = {}
        for eh in range(EH):
            for jq in range(JB):
                qb = JB * sup + jq
                nk = qb + 1
                po = ps_o.tile([P, TS], f32)
                if eh == 0:
                    pos = ps_sum.tile([P, 2], f32)
                for k in range(nk):
                    lhsT = pt_parts[k // QB][:, k % QB, ts(jq, P)]
                    nc.tensor.matmul(
                        po[:],
                        lhsT,
                        V[:, k, ts(eh, TS)],
                        start=(k == 0),
                        stop=(k == nk - 1),
                    )
                    if eh == 0:
                        nc.tensor.matmul(
                            pos[:],
                            lhsT,
                            ones_col[:],
                            start=(k == 0),
                            stop=(k == nk - 1),
                        )
                if eh == 0:
                    rs = rs_pool.tile(
                        [P, 1], f32, tag="rs", name=f"rs_{sup}_{jq}"
                    )
                    nc.vector.reciprocal(rs[:], pos[:, 0:1])
                    rss[jq] = rs
                ost = ostg.tile([P, TS], f32, tag="ostage")
                nc.scalar.activation(
                    ost[:], po[:], AF.Copy, scale=rss[jq][:]
                )
                seng = nc.scalar if jq % 2 == 0 else nc.sync
                seng.dma_start(out[ts(qb, P), ts(eh, TS)], ost[:])

    ps_sum.release()
    ps_o.release()
    ps_s.release()


def build_program():
    from contextlib import ExitStack

    import concourse.bacc as bacc
    import concourse.tile as tile
    from concourse import mybir

    nc = bacc.Bacc("TRN2", target_bir_lowering=False, debug=False)
    f32 = mybir.dt.float32
    bf16 = mybir.dt.bfloat16
    xt = nc.dram_tensor("xt16", [D, T], bf16, kind="ExternalInput").ap()
    wqp = nc.dram_tensor(
        "wq16", [P, EC, DC, P], bf16, kind="ExternalInput"
    ).ap()
    wkp = nc.dram_tensor(
        "wk16", [P, EC, DC, P], bf16, kind="ExternalInput"
    ).ap()
    wvp = nc.dram_tensor(
        "wv16", [P, EH, DC, TS], bf16, kind="ExternalInput"
    ).ap()
    out = nc.dram_tensor("out", [T, E], f32, kind="ExternalOutput").ap()

    with tile.TileContext(nc) as tc:
        with ExitStack() as ctx:
            _attention_kernel(ctx, tc, out, xt, wqp, wkp, wvp)
    nc.compile()
    return nc


def kernel(x, Wq, Wk, Wv, _trace=False):
    import ml_dtypes
    from concourse.bass_utils import run_bass_kernel_spmd

    x = np.ascontiguousarray(np.asarray(x), dtype=np.float32)
    Wq = np.ascontiguousarray(np.asarray(Wq), dtype=np.float32)
    Wk = np.ascontiguousarray(np.asarray(Wk), dtype=np.float32)
    Wv = np.ascontiguousarray(np.asarray(Wv), dtype=np.float32)
    assert x.shape == (N_CORES, T, D), x.shape

    bf = ml_dtypes.bfloat16
    # host-side input prep (sharding + layout): x^T per core in bf16;
    # weights laid out so each SBUF weight tile is one contiguous DMA.
    xts = [np.ascontiguousarray(x[b].T.astype(bf)) for b in range(N_CORES)]
    # [dp, eb, dc, ep]: tile [dp, dc, ep] contiguous per (eb)
    wq16 = np.ascontiguousarray(
        Wq.reshape(DC, P, EC, P).transpose(1, 2, 0, 3).astype(bf)
    )
    wk16 = np.ascontiguousarray(
        Wk.reshape(DC, P, EC, P).transpose(1, 2, 0, 3).astype(bf)
    )
    # [dp, eh, dc, e-within-half]
    wv16 = np.ascontiguousarray(
        Wv.reshape(DC, P, EH, TS).transpose(1, 2, 0, 3).astype(bf)
    )

    nc = build_program()
    in_maps = [
        {"xt16": xts[b], "wq16": wq16, "wk16": wk16, "wv16": wv16}
        for b in range(N_CORES)
    ]
    last_err = None
    for attempt in range(3):
        try:
            res = run_bass_kernel_spmd(
                nc, in_maps, core_ids=list(range(N_CORES)), trace=_trace
            )
            break
        except Exception as e:  # transient device wedge: retry
            last_err = e
            import time

            time.sleep(5.0 * (attempt + 1))
    else:
        raise last_err
    out = np.stack([res.results[b]["out"] for b in range(N_CORES)], axis=0)
    if _trace:
        kernel.last_results = res
    return out


kernel.last_results = None


# revision 45
# speedup vs baseline: 1.0416x; 1.0416x over previous
"""Causal attention (QKV proj + softmax(QK^T/sqrt(d))V) on 8 TRN2 NeuronCores.

Sharding: data-parallel over batch (B=8, one batch element per core).
Host-side input prep (part of kernel()'s sharding step): x is cast to
bf16 and transposed to x^T [D,T]; Wq/Wk/Wv are cast to bf16 and laid
out so every weight tile is a single contiguous DMA. All matmuls are
bf16: 1 cyc/col PE streaming + FWL fast weight load (f32r streams at
the same rate but its 4-byte LDWEIGHTS can't use FWL and stays
exposed; fp16 measured slower than bf16 on the MATMUL stream).

Per-core kernel:
  phase 1: per 512-wide t-slice: Q then K projections (weights fully
           resident in SBUF), evicted to resident Q^T / K^T tiles (no
           DRAM roundtrip); then V = x @ Wv into a resident V tile.
  phase 2: per 512-wide query supertile: S^T = K Q^T (probs produced
           directly in the lhsT layout P@V needs), diagonal band
           trimmed to the causal range, exp on ACT with fused
           1/sqrt(D) scale, 128x128 causal mask on the diagonal block
           only, P@V with interleaved ones-matmul row sums on PE,
           reciprocal normalize, store.

DMA queues (learned the hard way): sync(HWDGE) = x^T loads - latency
critical; gpsimd(SWDGE) = weight loads - bulk, latency tolerant;
scalar(HWDGE) = a few early x^T assists + output stores.
"""

import numpy as np

T = 2048
D = 1024
E = 1024
N_CORES = 8
P = 128
TS = 512  # t-slice / supertile width
SCALE = 1.0 / 32.0  # 1/sqrt(D)

DC = D // P  # 8 d-chunks
EC = E // P  # 8 e-chunks
TB = T // P  # 16 t-blocks of 128
NTS = T // TS  # 4 t-slices of 512
JB = TS // P  # 4 q-blocks per supertile
QB = TB // 4  # pT quarter size in k-blocks
EH = E // TS  # 2 e-halves


def _attention_kernel(ctx, tc, out, xt, wqp, wkp, wvp):
    from concourse import mybir
    from concourse.bass import ts

    nc = tc.nc
    f32 = mybir.dt.float32
    bf16 = mybir.dt.bfloat16
    AF = mybir.ActivationFunctionType

    # ---- left-side SBUF pools ----
    const = ctx.enter_context(tc.tile_pool(name="const", bufs=1))
    ones_f32 = const.tile([P, 2], f32)
    nc.vector.memset(ones_f32[:], 1.0)
    ones_col = const.tile([P, 2], bf16)
    nc.vector.tensor_copy(ones_col[:], ones_f32[:])
    # warm the ACT exp table set at program start (off the critical path)
    exp_warm = const.tile([P, 2], f32)
    nc.scalar.activation(exp_warm[:], ones_f32[:], AF.Exp)

    kt_pool = ctx.enter_context(tc.tile_pool(name="ktres", bufs=1))
    KT = kt_pool.tile([P, EC, T], bf16)  # K^T[e, t], e = ec*128 + ep
    qt_pool = ctx.enter_context(tc.tile_pool(name="qtres", bufs=1))
    QT = qt_pool.tile([P, EC, T], bf16)  # Q^T[e, t], resident

    # ---- right-side work pools ----
    tc.swap_default_side()
    xv_pool = ctx.enter_context(tc.tile_pool(name="xv", bufs=1))
    xv = xv_pool.tile([P, DC, T], bf16)  # x^T[d, t]: [dp, dc, t]
    v_pool = ctx.enter_context(tc.tile_pool(name="vres", bufs=1))
    V = v_pool.tile([P, TB, E], bf16)  # V[t, e]: [tl, tb, e]
    wres_pool = tc.alloc_tile_pool(name="wres", bufs=16)
    wvh_pool = tc.alloc_tile_pool(name="wvh", bufs=2)
    tc.swap_default_side()

    # ---- PSUM pool for phase 1 ----
    ps_proj = tc.alloc_tile_pool(name="ps_proj", bufs=8, space="PSUM")

    # ---- x^T streaming loads: per (t-slice, d-chunk), sync queue ----
    # (consumption order; first Q matmul can start after 8 chunks)
    def load_xt_tslice(tsl):
        for dc in range(DC):
            eng = nc.scalar if (tsl == 0 and dc % 2 == 1) else nc.sync
            eng.dma_start(
                xv[:, dc, ts(tsl, TS)], xt[ts(dc, P), ts(tsl, TS)]
            )

    load_xt_tslice(0)

    # ---- resident bf16 weight tiles (single contiguous DMAs) ----
    wr_q, wr_k = [], []
    for w_ap, lst, nm in ((wqp, wr_q, "q"), (wkp, wr_k, "k")):
        for eb in range(EC):
            wr = wres_pool.tile(
                [P, DC, P], bf16, tag="wres", name=f"wr{nm}_{eb}"
            )
            nc.gpsimd.dma_start(wr[:], w_ap[:, eb])
            lst.append(wr)
    wvhs = []
    for eh in range(EH):
        wvh = wvh_pool.tile([P, DC, TS], bf16, tag="wvh", name=f"wvh_{eh}")
        nc.gpsimd.dma_start(wvh[:], wvp[:, eh])
        wvhs.append(wvh)

    # t-slices 1-3: one large load per d-chunk (fewer queue ops)
    for dc in range(DC):
        nc.sync.dma_start(
            xv[:, dc, TS:T], xt[ts(dc, P), TS:T]
        )

    # 128x128 causal mask for the diagonal blocks of S^T: keep f >= p
    # (p = key partition, f = query col within the block).
    mask_pool = ctx.enter_context(tc.tile_pool(name="maskp", bufs=1))
    mask_f32 = mask_pool.tile([P, P], f32)
    nc.vector.memset(mask_f32[:], 1.0)
    nc.gpsimd.affine_select(
        out=mask_f32[:],
        in_=mask_f32[:],
        compare_op=mybir.AluOpType.is_ge,
        fill=0.0,
        base=0,
        pattern=[[1, P]],
        channel_multiplier=-1,
    )
    mask128 = mask_pool.tile([P, P], bf16)
    nc.vector.tensor_copy(mask128[:], mask_f32[:])


    # ===== phase 1a: per t-slice, Q then K projections =====
    for tsl in range(NTS):
        for wr_lst, dst in ((wr_q, QT), (wr_k, KT)):
            for eb in range(EC):
                pp = ps_proj.tile([P, TS], f32)
                for dc in range(DC):
                    nc.tensor.matmul(
                        pp[:],
                        wr_lst[eb][:, dc, :],
                        xv[:, dc, ts(tsl, TS)],
                        start=(dc == 0),
                        stop=(dc == DC - 1),
                    )
                if eb % 2 == 0:
                    nc.vector.tensor_copy(dst[:, eb, ts(tsl, TS)], pp[:])
                else:
                    nc.scalar.copy(dst[:, eb, ts(tsl, TS)], pp[:])

    # ===== phase 1b: V = x @ Wv into the resident V tile =====
    for tb in range(TB):
        for eh in range(EH):
            pp = ps_proj.tile([P, TS], f32)
            for dc in range(DC):
                nc.tensor.matmul(
                    pp[:],
                    xv[:, dc, ts(tb, P)],
                    wvhs[eh][:, dc, :],
                    start=(dc == 0),
                    stop=(dc == DC - 1),
                )
            if eh == 0:
                nc.vector.tensor_copy(V[:, tb, ts(eh, TS)], pp[:])
            else:
                nc.scalar.copy(V[:, tb, ts(eh, TS)], pp[:])

    wvh_pool.release()
    wres_pool.release()
    ps_proj.release()

    # ================= phase 2: attention =================
    ps_s = tc.alloc_tile_pool(name="ps_s", bufs=4, space="PSUM")
    ps_o = tc.alloc_tile_pool(name="ps_o", bufs=2, space="PSUM")
    ps_sum = tc.alloc_tile_pool(name="ps_sum", bufs=2, space="PSUM")

    tc.swap_default_side()
    pt_pool = ctx.enter_context(tc.tile_pool(name="pt", bufs=5))
    rs_pool = ctx.enter_context(tc.tile_pool(name="rs", bufs=8))
    ostg = ctx.enter_context(tc.tile_pool(name="ostg", bufs=3))
    tc.swap_default_side()

    for sup in range(NTS):
        nkb = JB * sup + JB  # key blocks 0..nkb-1
        pt_parts = [
            pt_pool.tile([P, QB, TS], bf16, tag="pt", name=f"ptp_{sup}_0")
        ]

        # --- S^T blocks + exp + causal mask (diagonal band trimmed) ---
        for k in range(nkb):
            j = k - JB * sup  # >= 0 on the diagonal band
            lo = max(0, j) * P  # first causal query col in this supertile
            ssp = ps_s.tile([P, TS], f32)
            for ec in range(EC):
                nc.tensor.matmul(
                    ssp[:, lo:TS],
                    KT[:, ec, ts(k, P)],
                    QT[:, ec, sup * TS + lo : (sup + 1) * TS],
                    start=(ec == 0),
                    stop=(ec == EC - 1),
                )
            if k // QB >= len(pt_parts):
                pt_parts.append(
                    pt_pool.tile(
                        [P, QB, TS], bf16, tag="pt",
                        name=f"ptp_{sup}_{k // QB}",
                    )
                )
            pk = pt_parts[k // QB][:, k % QB, :]
            nc.scalar.activation(
                pk[:, lo:TS], ssp[:, lo:TS], AF.Exp, scale=SCALE
            )
            if j >= 0:
                nc.vector.tensor_mul(
                    pk[:, lo : lo + P], pk[:, lo : lo + P], mask128[:]
                )

        # --- P @ V (+ row sums interleaved in eh=0), normalize, store ---
        rss = {}
        for eh in range(EH):
            for jq in range(JB):
                qb = JB * sup + jq
                nk = qb + 1
                po = ps_o.tile([P, TS], f32)
                if eh == 0:
                    pos = ps_sum.tile([P, 2], f32)
                for k in range(nk):
                    lhsT = pt_parts[k // QB][:, k % QB, ts(jq, P)]
                    nc.tensor.matmul(
                        po[:],
                        lhsT,
                        V[:, k, ts(eh, TS)],
                        start=(k == 0),
                        stop=(k == nk - 1),
                    )
                    if eh == 0:
                        nc.tensor.matmul(
                            pos[:],
                            lhsT,
                            ones_col[:],
                            start=(k == 0),
                            stop=(k == nk - 1),
                        )
                if eh == 0:
                    rs = rs_pool.tile(
                        [P, 1], f32, tag="rs", name=f"rs_{sup}_{jq}"
                    )
                    nc.vector.reciprocal(rs[:], pos[:, 0:1])
                    rss[jq] = rs
                ost = ostg.tile([P, TS], f32, tag="ostage")
                nc.scalar.activation(
                    ost[:], po[:], AF.Copy, scale=rss[jq][:]
                )
                seng = nc.scalar if jq % 2 == 0 else nc.sync
                seng.dma_start(out[ts(qb, P), ts(eh, TS)], ost[:])

    ps_sum.release()
    ps_o.release()
    ps_s.release()


def build_program():
    from contextlib import ExitStack

    import concourse.bacc as bacc
    import concourse.tile as tile
    from concourse import mybir

    nc = bacc.Bacc("TRN2", target_bir_lowering=False, debug=False)
    f32 = mybir.dt.float32
    bf16 = mybir.dt.bfloat16
    xt = nc.dram_tensor("xt16", [D, T], bf16, kind="ExternalInput").ap()
    wqp = nc.dram_tensor(
        "wq16", [P, EC, DC, P], bf16, kind="ExternalInput"
    ).ap()
    wkp = nc.dram_tensor(
        "wk16", [P, EC, DC, P], bf16, kind="ExternalInput"
    ).ap()
    wvp = nc.dram_tensor(
        "wv16", [P, EH, DC, TS], bf16, kind="ExternalInput"
    ).ap()
    out = nc.dram_tensor("out", [T, E], f32, kind="ExternalOutput").ap()

    with tile.TileContext(nc) as tc:
        with ExitStack() as ctx:
            _attention_kernel(ctx, tc, out, xt, wqp, wkp, wvp)
    nc.compile()
    return nc


def kernel(x, Wq, Wk, Wv, _trace=False):
    import ml_dtypes
    from concourse.bass_utils import run_bass_kernel_spmd

    x = np.ascontiguousarray(np.asarray(x), dtype=np.float32)
    Wq = np.ascontiguousarray(np.asarray(Wq), dtype=np.float32)
    Wk = np.ascontiguousarray(np.asarray(Wk), dtype=np.float32)
    Wv = np.ascontiguousarray(np.asarray(Wv), dtype=np.float32)
    assert x.shape == (N_CORES, T, D), x.shape

    bf = ml_dtypes.bfloat16
    # host-side input prep (sharding + layout): x^T per core in bf16;
    # weights laid out so each SBUF weight tile is one contiguous DMA.
    xts = [np.ascontiguousarray(x[b].T.astype(bf)) for b in range(N_CORES)]
    # [dp, eb, dc, ep]: tile [dp, dc, ep] contiguous per (eb)
    wq16 = np.ascontiguousarray(
        Wq.reshape(DC, P, EC, P).transpose(1, 2, 0, 3).astype(bf)
    )
    wk16 = np.ascontiguousarray(
        Wk.reshape(DC, P, EC, P).transpose(1, 2, 0, 3).astype(bf)
    )
    # [dp, eh, dc, e-within-half]
    wv16 = np.ascontiguousarray(
        Wv.reshape(DC, P, EH, TS).transpose(1, 2, 0, 3).astype(bf)
    )

    nc = build_program()
    in_maps = [
        {"xt16": xts[b], "wq16": wq16, "wk16": wk16, "wv16": wv16}
        for b in range(N_CORES)
    ]
    last_err = None
    for attempt in range(3):
        try:
            res = run_bass_kernel_spmd(
                nc, in_maps, core_ids=list(range(N_CORES)), trace=_trace
            )
            break
        except Exception as e:  # transient device wedge: retry
            last_err = e
            import time

            time.sleep(5.0 * (attempt + 1))
    else:
        raise last_err
    out = np.stack([res.results[b]["out"] for b in range(N_CORES)], axis=0)
    if _trace:
        kernel.last_results = res
    return out


kernel.last_results = None
